# revision 43
# baseline (speedup 1.0000x reference)
"""Trainium2 Bass kernel for a Mamba block (LayerNorm -> in_proj -> causal
depthwise conv1d + SiLU -> selective scan (SSM) -> gate -> out_proj).

Full inputs (B=8, L=2048, d_model=128) are sharded batch-parallel across the
8 NeuronCores (one batch element per core, no collectives). The second
reference output, `residual`, equals the input `x` and is returned host-side.

Per-core pipeline (channel-on-partition, time-on-free layout, fp16 compute):
  - LN stats on VectorE in (t, d) tiles; normalize on ScalarE with per-
    partition scale/bias; LN affine folded into in_proj weights on host.
  - TensorE transposes to (d, t); in_proj / x_proj / dt_proj matmuls.
    SiLU(z) is fused into the in_proj PSUM->SBUF copy.
  - conv1d as per-partition tensor_scalar/scalar_tensor_tensor taps.
  - selective scan processes STATE PAIRS in single ops: dA/dBx/h tiles are
    [128, 2*L] with two n-segments back to back; forcing dA=0 at the second
    segment start resets the scan state (h0=0), so one tensor_tensor_scan
    computes both states. B/C rows of a pair arrive as ONE [128, 2, 2, L]
    broadcast DMA (DRAM round trip spreads the replicated read over HBM),
    alternating between the two HWDGE rings (sync/scalar).
  - y = sum_n C_n * h_n: C-multiply on VectorE, n-reduction as identity
    matmuls accumulated in a single 4-bank PSUM tile.
  - skip term + gate (one [128, 2, L] op), out_proj with y as the matmul
    stationary so the output lands directly in (t, d_model) layout.
"""
import os
import numpy as np

D_MODEL, D_INNER, D_STATE, D_CONV, DT_RANK = 128, 256, 16, 4, 8
L = 2048
N_CORES = 8
NT = L // 128          # 16 t-tiles of 128
NC4 = L // 512         # 4 t-chunks of 512

_cache = {}


def _build(reps=1, legalize=True, pool_hc_every=0, scan_bufs=3,
           n_pairs=D_STATE // 2, bcast_bufs=2, cb_bufs=2, da_bufs=3,
           ring_alt=True, hst_bufs=1, probe=""):
    import concourse.bass as bass
    import concourse.tile as tile
    from concourse import mybir
    from concourse import masks

    f32 = mybir.dt.float32
    f16 = mybir.dt.float16
    ts = bass.ts
    Alu = mybir.AluOpType
    Act = mybir.ActivationFunctionType

    nc = bass.Bass()

    # ---- DRAM I/O (per core) ----
    x_d = nc.dram_tensor("x", [L, D_MODEL], f32, kind="ExternalInput")
    w1t_d = nc.dram_tensor("w1t", [D_MODEL, 2 * D_INNER], f16, kind="ExternalInput")
    bias1_d = nc.dram_tensor("bias1", [D_MODEL, 4], f32, kind="ExternalInput")
    xpt_d = nc.dram_tensor("xpt", [128, 2, 96], f16, kind="ExternalInput")
    dtpt_d = nc.dram_tensor("dtpt", [DT_RANK, D_INNER], f16, kind="ExternalInput")
    dtb_d = nc.dram_tensor("dtb", [128, 2], f32, kind="ExternalInput")
    convw_d = nc.dram_tensor("convw", [128, 2, D_CONV], f32, kind="ExternalInput")
    convb_d = nc.dram_tensor("convb", [128, 2], f32, kind="ExternalInput")
    A_d = nc.dram_tensor("A", [128, 2, D_STATE], f32, kind="ExternalInput")
    Dp_d = nc.dram_tensor("Dp", [128, 2], f32, kind="ExternalInput")
    w2t_d = nc.dram_tensor("w2t", [128, 2, D_MODEL], f16, kind="ExternalInput")
    out_d = nc.dram_tensor("out", [L, D_MODEL], f32, kind="ExternalOutput")
    bc_d = nc.dram_tensor("bc_scratch", [D_STATE, 2, L], f16, kind="Internal")

    with tile.TileContext(nc) as tc:
        with (
            tc.tile_pool(name="singles", bufs=1) as singles,
            tc.tile_pool(name="big", bufs=1) as big,
            tc.tile_pool(name="ln", bufs=4) as lnp,
            tc.tile_pool(name="scan", bufs=scan_bufs) as scanp,
            tc.tile_pool(name="bcast", bufs=bcast_bufs) as bcastp,
            tc.tile_pool(name="pp", bufs=2, space="PSUM") as pp,
            tc.tile_pool(name="ppy", bufs=1, space="PSUM") as ppy,
        ):
            # ---- load weights ----
            w1t = singles.tile([128, 2 * D_INNER], f16)
            nc.sync.dma_start(w1t, w1t_d[:])
            bias1 = singles.tile([128, 4], f32)
            nc.sync.dma_start(bias1, bias1_d[:])
            xpt = singles.tile([128, 2, 96], f16)
            nc.sync.dma_start(xpt, xpt_d[:])
            dtpt = singles.tile([DT_RANK, D_INNER], f16)
            nc.sync.dma_start(dtpt, dtpt_d[:])
            dtb = singles.tile([128, 2], f32)
            nc.sync.dma_start(dtb, dtb_d[:])
            convw = singles.tile([128, 2, D_CONV], f32)
            nc.sync.dma_start(convw, convw_d[:])
            convb = singles.tile([128, 2], f32)
            nc.sync.dma_start(convb, convb_d[:])
            A_sb = singles.tile([128, 2, D_STATE], f32)
            nc.sync.dma_start(A_sb, A_d[:])
            Dp = singles.tile([128, 2], f32)
            nc.sync.dma_start(Dp, Dp_d[:])
            w2t = singles.tile([128, 2, D_MODEL], f16)
            nc.sync.dma_start(w2t, w2t_d[:])
            ident = singles.tile([128, 128], f16)
            masks.make_identity(nc, ident[:])
            eps = singles.tile([128, 1], f32)
            nc.vector.memset(eps, 1e-5)

            for _rep in range(reps):
                # ---- load x: (2048, 128) -> (128 part, 16, 128) ----
                x_sb = big.tile([128, NT, D_MODEL], f32, tag="xio")
                nc.sync.dma_start(x_sb, x_d.rearrange("(i p) d -> p i d", p=128))

                # ---- LayerNorm (stats per t-row; t on partitions) ----
                xn16 = big.tile([128, NT, D_MODEL], f16)
                for i in range(NT):
                    st = lnp.tile([128, 6], f32, tag="st")
                    nc.vector.bn_stats(st, x_sb[:, i, :])
                    mv = lnp.tile([128, 2], f32, tag="mv")
                    nc.vector.bn_aggr(mv, st)
                    sd = lnp.tile([128, 1], f32, tag="sd")
                    nc.scalar.activation(sd, mv[:, 1:2], Act.Sqrt, bias=eps[:])
                    rstd = lnp.tile([128, 1], f32, tag="rstd")
                    nc.vector.reciprocal(rstd, sd)
                    nmr = lnp.tile([128, 1], f32, tag="nmr")
                    # nmr = -(mu * rstd)
                    nc.vector.tensor_scalar(nmr, mv[:, 0:1], rstd, -1.0,
                                            op0=Alu.mult, op1=Alu.mult)
                    nc.scalar.activation(xn16[:, i, :], x_sb[:, i, :], Act.Identity,
                                         bias=nmr, scale=rstd)

                # ---- transpose xn -> (d_model, t) ----
                xnT = big.tile([128, L], f16)
                for i in range(NT):
                    pt = pp.tile([128, 128], f16, tag="pp")
                    nc.tensor.transpose(pt, xn16[:, i, :], ident)
                    nc.scalar.copy(xnT[:, ts(i, 128)], pt)

                # ---- in_proj: xz = W1eff @ xn (c on partitions) ----
                # xm (c<256) -> padded conv buffer; z -> silu fused into the
                # PSUM->SBUF copy.
                # data lives at [4, 4+L); tap k reads [k+1, k+1+L)
                xm_pad = big.tile([128, 2, L + D_CONV], f16, tag="xm_pad")
                sz = big.tile([128, 2, L], f16, tag="sz")
                nc.vector.memset(xm_pad[:, :, 0:D_CONV], 0.0)
                for co in range(4):
                    for cp in range(2):
                        pz = pp.tile([128, 1024], f32, tag="pp")
                        for q in range(2):
                            nc.tensor.matmul(pz[:, ts(q, 512)], w1t[:, ts(co, 128)],
                                             xnT[:, ts(cp * 2 + q, 512)],
                                             start=True, stop=True)
                        if co < 2:
                            nc.scalar.activation(
                                xm_pad[:, co,
                                       D_CONV + cp * 1024:D_CONV + (cp + 1) * 1024],
                                pz, Act.Identity, bias=bias1[:, co:co + 1])
                        else:
                            nc.scalar.activation(sz[:, co - 2, ts(cp, 1024)], pz,
                                                 Act.Silu, bias=bias1[:, co:co + 1])

                # ---- causal depthwise conv + SiLU ----
                c3 = big.tile([128, 2, L], f16, tag="c3")
                for h in range(2):
                    c0 = lnp.tile([128, L], f16, tag="conv0", bufs=1)
                    nc.vector.tensor_scalar(c0, xm_pad[:, h, 1:1 + L],
                                            convw[:, h, 0:1], convb[:, h:h + 1],
                                            op0=Alu.mult, op1=Alu.add)
                    c1 = lnp.tile([128, L], f16, tag="conv1", bufs=1)
                    nc.vector.scalar_tensor_tensor(c1, xm_pad[:, h, 2:2 + L],
                                                   convw[:, h, 1:2], c0,
                                                   op0=Alu.mult, op1=Alu.add)
                    c2 = lnp.tile([128, L], f16, tag="conv0", bufs=1)
                    nc.vector.scalar_tensor_tensor(c2, xm_pad[:, h, 3:3 + L],
                                                   convw[:, h, 2:3], c1,
                                                   op0=Alu.mult, op1=Alu.add)
                    nc.vector.scalar_tensor_tensor(c3[:, h, :], xm_pad[:, h, 4:4 + L],
                                                   convw[:, h, 3:4], c2,
                                                   op0=Alu.mult, op1=Alu.add)
                xc = big.tile([128, 2, L], f16, tag="xc")
                nc.scalar.activation(xc, c3, Act.Silu)

                # ---- x_proj: dbc = x_proj_w @ xc  (40, t) ----
                dt_sb = big.tile([DT_RANK, L], f16)
                B_sb = big.tile([D_STATE, L], f16)
                C_sb = big.tile([D_STATE, L], f16)
                for tn in range(NC4):
                    pd = pp.tile([96, 512], f32, tag="pp")
                    nc.tensor.matmul(pd, xpt[:, 0, :], xc[:, 0, ts(tn, 512)],
                                     start=True, stop=False)
                    nc.tensor.matmul(pd, xpt[:, 1, :], xc[:, 1, ts(tn, 512)],
                                     start=False, stop=True)
                    nc.scalar.copy(dt_sb[:, ts(tn, 512)], pd[0:DT_RANK, :])
                    nc.scalar.copy(B_sb[:, ts(tn, 512)], pd[32:32 + D_STATE, :])
                    nc.scalar.copy(C_sb[:, ts(tn, 512)], pd[64:64 + D_STATE, :])
                # (n, B/C, L) layout so one DMA per n-pair broadcasts 4 rows
                nc.sync.dma_start(bc_d[:, 0, :], B_sb)
                nc.sync.dma_start(bc_d[:, 1, :], C_sb)

                # ---- delta = softplus(dt_proj_w @ dt + b)  (d on partitions) ----
                delta = big.tile([128, 2, L], f16, tag="delta")
                for h in range(2):
                    for cp in range(2):
                        pdl = pp.tile([128, 1024], f32, tag="pp")
                        for q in range(2):
                            nc.tensor.matmul(pdl[:, ts(q, 512)], dtpt[:, ts(h, 128)],
                                             dt_sb[:, ts(cp * 2 + q, 512)],
                                             start=True, stop=True)
                        # walrus act-tables here lack Softplus; compose it
                        edl = lnp.tile([128, 1024], f16, tag="edl", bufs=2)
                        nc.scalar.activation(edl, pdl, Act.Exp, bias=dtb[:, h:h + 1])
                        nc.scalar.activation(delta[:, h, ts(cp, 1024)], edl,
                                             Act.Ln, bias=1.0)

                # ---- u = delta * xc ----
                u = big.tile([128, 2, L], f16, tag="u")
                nc.vector.tensor_tensor(u, delta, xc, op=Alu.mult)

                # ---- selective scan (state pairs) + y reduction ----
                # yd reuses c3's buffer (dead after the xc silu)
                yd16 = big.tile([128, 2, L], f16, tag="c3", name="yd")
                for h in range(2):
                    if n_pairs == 0:
                        # timing-attribution mode: skip the scan loop entirely
                        nc.vector.tensor_scalar(
                            yd16[:, h, :], xc[:, h, :], Dp[:, h:h + 1], 0.0,
                            op0=Alu.mult, op1=Alu.add)
                        continue
                    py = ppy.tile([128, L], f32, tag="py")
                    if "static_bc" in probe:
                        Bb0 = bcastp.tile([128, 2, L], f16, tag="Bb")
                        nc.sync.dma_start(
                            Bb0, bc_d[None, 0:2, 0, :].broadcast_to([128, 2, L]))
                        Cb0 = bcastp.tile([128, 2, L], f16, tag="Cb", bufs=cb_bufs)
                        nc.sync.dma_start(
                            Cb0, bc_d[None, 0:2, 1, :].broadcast_to([128, 2, L]))
                    if "static_da" in probe:
                        dA0 = scanp.tile([128, 2 * L], f16, tag="dA", bufs=da_bufs)
                        nc.scalar.activation(dA0[:, 0:L], delta[:, h, :], Act.Exp,
                                             scale=A_sb[:, h, 0:1])
                        nc.scalar.activation(dA0[:, L:2 * L], delta[:, h, :], Act.Exp,
                                             scale=A_sb[:, h, 1:2])
                        nc.scalar.mul(dA0[:, L:L + 1], dA0[:, L:L + 1], 0.0)
                    for p_ in range(n_pairs):
                        n0 = 2 * p_
                        # both B and C rows of the pair in ONE broadcast DMA,
                        # alternating between the two HWDGE rings
                        if "static_bc" in probe:
                            Bb, Cb = Bb0, Cb0
                        else:
                            BC = bcastp.tile([128, 2, 2, L], f16, tag="Bb")
                            deng = nc.scalar if (ring_alt and p_ % 2) else nc.sync
                            deng.dma_start(
                                BC, bc_d[None, n0:n0 + 2, :, :].broadcast_to(
                                    [128, 2, 2, L]))
                            Bb = BC[:, :, 0, :]
                            Cb = BC[:, :, 1, :]
                        if "static_da" in probe:
                            dA = dA0
                        else:
                            dA = scanp.tile([128, 2 * L], f16, tag="dA",
                                            bufs=da_bufs)
                            for j in range(2):
                                nc.scalar.activation(
                                    dA[:, ts(j, L)], delta[:, h, :], Act.Exp,
                                    scale=A_sb[:, h, n0 + j:n0 + j + 1])
                            # reset the scan state at the second segment start;
                            # on ScalarE so it orders behind the exps without a
                            # cross-engine semaphore
                            nc.scalar.mul(dA[:, L:L + 1], dA[:, L:L + 1], 0.0)
                        dBx = scanp.tile([128, 2, L], f16, tag="dBx")
                        nc.vector.tensor_tensor(
                            dBx, u[:, h:h + 1, :].broadcast_to([128, 2, L]),
                            Bb, op=Alu.mult)
                        hst = scanp.tile([128, 2 * L], f16, tag="hst",
                                         bufs=hst_bufs)
                        if "no_scan" in probe:
                            nc.vector.tensor_tensor(
                                hst, dA, dBx.rearrange("p j l -> p (j l)"),
                                op=Alu.mult)
                        else:
                            nc.vector.tensor_tensor_scan(
                                hst, dA, dBx.rearrange("p j l -> p (j l)"), 0.0,
                                op0=Alu.mult, op1=Alu.add)
                        # hc reuses the dBx tag: dBx is dead once the scan read it
                        hc = scanp.tile([128, 2, L], f16, tag="dBx")
                        hceng = nc.gpsimd if (
                            pool_hc_every and p_ % pool_hc_every == pool_hc_every - 1
                        ) else nc.vector
                        hceng.tensor_tensor(hc, hst.rearrange("p (j l) -> p j l", j=2),
                                            Cb, op=Alu.mult)
                        if "no_pe" not in probe:
                            for j in range(2):
                                for c in range(NC4):
                                    nc.tensor.matmul(
                                        py[:, ts(c, 512)], ident,
                                        hc[:, j, ts(c, 512)],
                                        start=(p_ == 0 and j == 0),
                                        stop=(p_ == n_pairs - 1 and j == 1))
                    if "no_pe" in probe:
                        for c in range(NC4):
                            nc.tensor.matmul(py[:, ts(c, 512)], ident,
                                             hc[:, 0, ts(c, 512)],
                                             start=True, stop=True)
                    # yd = (xc * D) + y_psum
                    nc.vector.scalar_tensor_tensor(
                        yd16[:, h, :], xc[:, h, :], Dp[:, h:h + 1], py,
                        op0=Alu.mult, op1=Alu.add)

                # ---- gate (one op over both halves) ----
                # yg reuses xm_pad's buffer (dead after the conv taps)
                yg_t = big.tile([128, 2, L + D_CONV], f16, tag="xm_pad", name="yg")
                yg16 = yg_t[:, :, 0:L]
                nc.vector.tensor_tensor(yg16, yd16, sz, op=Alu.mult)

                # ---- out_proj: out[t, dm] = sum_c yg[c, t] * w2t[c, dm] ----
                out_sb = big.tile([128, NT, D_MODEL], f32, tag="xio")
                for i in range(NT):
                    po = pp.tile([128, D_MODEL], f32, tag="pp")
                    nc.tensor.matmul(po, yg_t[:, 0, ts(i, 128)], w2t[:, 0, :],
                                     start=True, stop=False)
                    nc.tensor.matmul(po, yg_t[:, 1, ts(i, 128)], w2t[:, 1, :],
                                     start=False, stop=True)
                    nc.scalar.copy(out_sb[:, i, :], po)

                nc.sync.dma_start(out_d.rearrange("(i p) d -> p i d", p=128), out_sb)

    if legalize:
        _legalize_waits(nc)
    return nc


def _legalize_waits(nc):
    """This container's walrus codegen rejects instructions carrying more
    than one sync wait. Hoist extra waits onto preceding wait-only
    InstEventSemaphore instructions on the same engine (sequencers execute
    them in order, so the semantics are identical)."""
    from concourse import mybir

    fixid = [0]
    for fn in nc.m.functions:
        for blk in fn.blocks:
            out = []
            changed = False
            for ins in blk.instructions:
                si = getattr(ins, "sync_info", None)
                waits = list(si.on_wait) if si is not None and si.on_wait else []
                if len(waits) > 1:
                    for w in waits[:-1]:
                        fixid[0] += 1
                        nop = mybir.InstEventSemaphore(
                            name=f"I-waitfix-{fixid[0]}", ins=[], outs=[],
                            sync_info=mybir.SyncInfo(on_wait=[w], on_update=[]))
                        nop.engine = ins.engine
                        out.append(nop)
                    ins.sync_info = mybir.SyncInfo(
                        on_wait=[waits[-1]], on_update=list(si.on_update))
                    changed = True
                out.append(ins)
            if changed:
                blk.instructions = out


def _prep_inputs(x, norm_w, norm_b, in_proj_w, conv_w, conv_b, x_proj_w,
                 dt_proj_w, dt_proj_b, A_log, D, out_proj_w):
    """Host-side weight prep; returns per-core input maps."""
    f32 = np.float32
    f16 = np.float16
    W1eff = (in_proj_w.astype(f32) * norm_w.astype(f32)[None, :])
    w1t = np.ascontiguousarray(W1eff.T).astype(f16)                   # (128, 512)
    bias1 = (in_proj_w.astype(f32) @ norm_b.astype(f32))              # (512,)
    bias1 = np.ascontiguousarray(bias1.reshape(4, 128).T).astype(f32)  # (128, 4)
    xpw_pad = np.zeros((96, 256), f32)
    xpw_pad[0:8] = x_proj_w[0:8]
    xpw_pad[32:48] = x_proj_w[8:24]
    xpw_pad[64:80] = x_proj_w[24:40]
    xpt = np.ascontiguousarray(
        xpw_pad.T.reshape(2, 128, 96).transpose(1, 0, 2)).astype(f16)
    dtpt = np.ascontiguousarray(dt_proj_w.astype(f32).T).astype(f16)  # (8, 256)
    dtb = np.ascontiguousarray(dt_proj_b.astype(f32).reshape(2, 128).T)
    convw = np.ascontiguousarray(
        conv_w.astype(f32).reshape(2, 128, D_CONV).transpose(1, 0, 2))
    convb = np.ascontiguousarray(conv_b.astype(f32).reshape(2, 128).T)
    A = (-np.exp(A_log.astype(f32)))
    A = np.ascontiguousarray(A.reshape(2, 128, D_STATE).transpose(1, 0, 2))
    Dp = np.ascontiguousarray(D.astype(f32).reshape(2, 128).T)
    w2t = np.ascontiguousarray(
        out_proj_w.astype(f32).T.reshape(2, 128, D_MODEL).transpose(1, 0, 2)).astype(f16)

    shared = dict(w1t=w1t, bias1=bias1, xpt=xpt, dtpt=dtpt, dtb=dtb,
                  convw=convw, convb=convb, A=A, Dp=Dp, w2t=w2t)
    in_maps = []
    for b in range(N_CORES):
        m = dict(shared)
        m["x"] = np.ascontiguousarray(x[b].astype(f32))
        in_maps.append(m)
    return in_maps


def kernel(**inputs):
    from concourse.bass_utils import run_bass_kernel_spmd

    if "nc" not in _cache:
        _cache["nc"] = _build()
    nc = _cache["nc"]

    x = np.asarray(inputs["x"])
    in_maps = _prep_inputs(**{k: np.asarray(v) for k, v in inputs.items()})
    res = run_bass_kernel_spmd(nc, in_maps, list(range(N_CORES)),
                               trace=bool(int(os.environ.get("KTRACE", "0"))))
    _cache["last_results"] = res
    out = np.stack([res.results[b]["out"] for b in range(N_CORES)]).astype(np.float32)
    residual = x.astype(np.float32).copy()
    return out, residual
